# revision 23
# baseline (speedup 1.0000x reference)
"""Trainium2 Bass kernel v2 for multi-head attention (dense transformer).

Reference computation (per batch element):
    qkv = x @ w_qkv                      # [N, 3C]
    q, k, v = split heads (H=12, HD=64); q *= HD**-0.5
    out = softmax(q k^T) v               # full [N, N] scores
    out = merge_heads(out) @ w_proj + b_proj

Distribution: pure data parallel over batch — 8 elements, 8 cores.

Cost-model-driven redesign vs v1: the CoreSim cost model charges a matmul
out_free_dim x 0.42ns regardless of K/M, so the v1 denominator matmuls
(768 x FD=512) cost a full 166us and the col-packed attnV another 166us.
v2 restructures:
  * attnV is FLIPPED: lhsT = at-tile [128 keys, 128 queries] (stationary),
    rhs = v_ext [128 keys, 65] where column 64 is ones -> out [128 q, 65]
    accumulated over the 16 key tiles.  M=128 doubles the output per
    instruction (FD=65 vs 512), and the softmax denominator rides along as
    column 64 for +1 cycle.  attnV+denominators: 332us -> 83us.
  * normalization becomes a per-partition tensor_scalar DIVIDE on DVE
    (queries live on partitions), killing v1's DMA-bounce broadcast.
  * out is produced in [token, channel] layout; 96 PE transposes (~5us)
    restore the lhsT layout for the final projection.
  * sweep order is quad-major (3 sweeps of 4 heads x full N), so the
    qkT/v prefetch for quad g+1 hides inside quad g's ACT-bound sweep and
    the proj/transpose work hides inside quad 2's sweep.
ScalarE (384 x FD=1024 exp, ~399us busy) is the design floor; everything
else is scheduled to keep it saturated.

PSUM budget (8 banks): scores 2x[128,1024]f32 (4) + attnV accum 7/7/2
slots of [128,65]f32 (3) + one shared spare bank for qk/v/proj/transpose
groups (1).
"""

import os

import numpy as np

import concourse.bass as bass
import concourse.mybir as mybir
from concourse import bacc, bass_utils
from concourse.masks import make_identity
from concourse.tile import TileContext

F32 = mybir.dt.float32
BF16 = mybir.dt.bfloat16
AF = mybir.ActivationFunctionType
ALU = mybir.AluOpType

B, N, C = 8, 2048, 768
H, HD = 12, 64
SCALE = HD ** -0.5  # folded into the exp activation
P = 128
NT = N // P          # 16 key tiles
CT = C // P          # 6 feature tiles
NCH = 4              # query chunks per sweep
QW = N // NCH        # 512
QS = QW // P         # 4 query subtiles per chunk


def build_nc() -> bass.Bass:
    nc = bacc.Bacc(None)
    x = nc.declare_dram_parameter("x", [N, C], F32, isOutput=False)
    w_qkv = nc.declare_dram_parameter("w_qkv", [C, 3 * C], F32, isOutput=False)
    w_proj = nc.declare_dram_parameter("w_proj", [C, C], F32, isOutput=False)
    b_proj = nc.declare_dram_parameter("b_proj", [C], F32, isOutput=False)
    out = nc.declare_dram_parameter("out", [N, C], F32, isOutput=True)

    with TileContext(nc) as tc:
        with (
            tc.tile_pool(name="const", bufs=1) as cpool,
            tc.tile_pool(name="dram", bufs=1, space="DRAM") as dpool,
            tc.tile_pool(name="at", bufs=10) as at_pool,
            tc.tile_pool(name="rcp", bufs=2) as rcp_pool,
            tc.tile_pool(name="fin", bufs=2) as fin_pool,
            tc.tile_pool(name="psc", bufs=2, space="PSUM") as psum_sc,
            tc.tile_pool(name="pacc", bufs=1, space="PSUM") as psum_acc,
            tc.tile_pool(name="psp", bufs=1, space="PSUM") as psum_sp,
        ):
            # ---- persistent SBUF tensors -------------------------------
            w_qkv_sb = cpool.tile([P, CT, 3 * C], BF16, tag="wqkv")
            wproj_sb = cpool.tile([P, CT, C], BF16, tag="wproj")
            b_bc = cpool.tile([P, C], F32, tag="bias")
            xT = cpool.tile([P, CT, N], BF16, tag="xT")
            qkT = cpool.tile([P, 12, N], BF16, tag="qkT")  # q pairs 0-5, k 6-11
            v4 = cpool.tile([P, NT, H, HD + 1], BF16, tag="v4")  # col 64 = ones
            out_sb = cpool.tile([P, NT, C], BF16, tag="outsb")  # [token, chan]
            outT = cpool.tile([P, CT, N], BF16, tag="outT")
            ident = cpool.tile([P, P], BF16, tag="ident")
            warm = cpool.tile([P, 1], F32, tag="warm")

            # ---- phase 0: act-table preload, input DMAs ----------------
            nc.vector.memset(warm[:], 0.0)
            nc.scalar.activation(warm[:], warm[:], AF.Exp)  # preload exp table
            make_identity(nc, ident)
            nc.vector.memset(v4[:, :, :, HD], 1.0)  # denominator ones columns

            # DMA transfers serialize on a single global device in the cost
            # model, so the startup chain is ordered by need: x casts, the
            # first quarter of the xT transposes (token chunk 0), the w_qkv
            # bands (gate the upfront qkT groups), then the remaining
            # transpose pieces ahead of their kT/qT deadlines.
            x_bf = dpool.tile([N, C], BF16)
            wq_r = w_qkv.rearrange("(o p) j -> p o j", p=P)
            for ct in range(CT):
                csl = slice(ct * P, (ct + 1) * P)
                nc.gpsimd.dma_start(out=x_bf[:, csl], in_=x[:, csl])
            for ct in range(CT):
                nc.gpsimd.dma_start(out=w_qkv_sb[:, ct, :], in_=wq_r[:, ct, :])
            for ct in range(CT):
                csl = slice(ct * P, (ct + 1) * P)
                nc.sync.dma_start_transpose(xT[:, ct, :], x_bf[:, csl])
            nc.sync.dma_start(out=b_bc[:], in_=b_proj[None, :].to_broadcast((P, C)))
            nc.gpsimd.dma_start(
                out=wproj_sb[:], in_=w_proj.rearrange("(o p) j -> p o j", p=P)
            )

            # ---- emit helpers ------------------------------------------
            def emit_qk_group(jt: int, c4: int, pool, tag: str):
                """qkT[:, jt, c4*512:...] = w_qk(jt)^T @ x^T chunk."""
                ps = pool.tile([P, QW], F32, tag=tag, name=f"qk{jt}_{c4}")
                for ct in range(CT):
                    nc.tensor.matmul(
                        ps[:, 0:QW],
                        lhsT=w_qkv_sb[:, ct, jt * P:(jt + 1) * P],
                        rhs=xT[:, ct, c4 * QW:(c4 + 1) * QW],
                        start=(ct == 0),
                        stop=(ct == CT - 1),
                    )
                nc.vector.tensor_copy(
                    out=qkT[:, jt, c4 * QW:(c4 + 1) * QW], in_=ps[:, 0:QW]
                )

            def emit_v_group(m: int, g: int):
                """v for key tile m, heads 4g..4g+3, natural [key, hd] layout."""
                ps = psum_sp.tile([P, QW], F32, tag="sp", name=f"v{g}_{m}")
                for ct in range(CT):
                    nc.tensor.matmul(
                        ps[:, 0:256],
                        lhsT=xT[:, ct, m * P:(m + 1) * P],
                        rhs=w_qkv_sb[:, ct, 2 * C + g * 256: 2 * C + (g + 1) * 256],
                        start=(ct == 0),
                        stop=(ct == CT - 1),
                    )
                nc.vector.tensor_copy(
                    out=v4[:, m, 4 * g:4 * (g + 1), 0:HD], in_=ps[:, 0:256]
                )

            def emit_transpose(nt_: int):
                """outT[:, :, nt] = out_sb[nt]^T via 6 PE-mode transposes."""
                tp = psum_sp.tile([P, 8, P], BF16, tag="sp", name=f"tp{nt_}")
                for ct in range(CT):
                    nc.tensor.transpose(
                        tp[:, ct, :], out_sb[:, nt_, ct * P:(ct + 1) * P], ident
                    )
                nsl = slice(nt_ * P, (nt_ + 1) * P)
                nc.vector.tensor_copy(out=outT[:, 0:4, nsl], in_=tp[:, 0:4, :])
                nc.vector.tensor_copy(out=outT[:, 4:6, nsl], in_=tp[:, 4:6, :])

            def emit_proj(nt_: int, slot_: int, pool, tag: str):
                """final[nt tile, eo:eo+ew] = outT^T @ w_proj + b."""
                eo, ew = ((0, QW), (QW, 256))[slot_]
                ps = pool.tile([P, QW], F32, tag=tag, name=f"pj{nt_}_{slot_}")
                for ct in range(CT):
                    nc.tensor.matmul(
                        ps[:, 0:ew],
                        lhsT=outT[:, ct, nt_ * P:(nt_ + 1) * P],
                        rhs=wproj_sb[:, ct, eo:eo + ew],
                        start=(ct == 0),
                        stop=(ct == CT - 1),
                    )
                fs = fin_pool.tile([P, QW], F32, tag="fin")
                nc.vector.tensor_tensor(
                    fs[:, 0:ew], ps[:, 0:ew], b_bc[:, eo:eo + ew], ALU.add
                )
                nc.sync.dma_start(
                    out=out[nt_ * P:(nt_ + 1) * P, eo:eo + ew], in_=fs[:, 0:ew]
                )

            # ---- spare-bank task plan per (quad, chunk) window ---------
            def QK(jt, c4):
                return lambda: emit_qk_group(jt, c4, psum_sp, "sp")

            def VG(g, m):
                return lambda: emit_v_group(m, g)

            def TR(nt_):
                return lambda: emit_transpose(nt_)

            def PJ(nt_, s_):
                return lambda: emit_proj(nt_, s_, psum_sp, "sp")

            # plan[(q, c)][m] = spare tasks emitted at iteration (q, c, m),
            # placed BEFORE that iteration's attnV and AFTER its lookahead
            # scores.  Deadlines: VG(g, m') by iteration m' of g's first
            # chunk; kT QK(jt, c4) one iteration before scores m=4*c4; next
            # chunk's qT one iteration before the window ends.
            plan = {
                # quad 0 sweep: own kT/qT tails + v heads 0-3; prefetch quad 1
                (0, 0): {0: [VG(0, 0)], 1: [VG(0, 1), QK(6, 1), QK(7, 1)],
                         2: [VG(0, 2)], 3: [VG(0, 3)],
                         4: [VG(0, 4), QK(6, 2)], 5: [VG(0, 5), QK(7, 2)],
                         6: [VG(0, 6)], 7: [VG(0, 7), QK(6, 3)],
                         8: [VG(0, 8)], 9: [VG(0, 9), QK(7, 3)],
                         10: [VG(0, 10)], 11: [VG(0, 11), QK(0, 1)],
                         12: [VG(0, 12)], 13: [VG(0, 13), QK(1, 1)],
                         14: [VG(0, 14)], 15: [VG(0, 15)]},
                (0, 1): {0: [QK(0, 2)], 2: [QK(1, 2)], 4: [QK(8, 0)],
                         6: [QK(9, 0)]},
                (0, 2): {0: [QK(0, 3)], 2: [QK(1, 3)], 4: [QK(8, 1)],
                         6: [QK(9, 1)]},
                (0, 3): {0: [QK(8, 2)], 2: [QK(9, 2)], 4: [QK(2, 0)],
                         6: [QK(3, 0)], 8: [VG(1, 0)], 10: [VG(1, 1)],
                         12: [VG(1, 2)], 14: [VG(1, 3)]},
                # quad 1 sweep
                (1, 0): {0: [QK(8, 3)], 1: [QK(9, 3)], 2: [VG(1, 4)],
                         3: [VG(1, 5)], 4: [VG(1, 6)], 5: [VG(1, 7)],
                         6: [VG(1, 8)], 7: [VG(1, 9)], 8: [VG(1, 10)],
                         9: [VG(1, 11)], 10: [VG(1, 12)], 11: [VG(1, 13)],
                         12: [VG(1, 14), QK(2, 1)], 13: [VG(1, 15), QK(3, 1)]},
                (1, 1): {0: [QK(2, 2)], 2: [QK(3, 2)], 4: [QK(10, 0)],
                         6: [QK(11, 0)]},
                (1, 2): {0: [QK(2, 3)], 2: [QK(3, 3)], 4: [QK(10, 1)],
                         6: [QK(11, 1)]},
                (1, 3): {0: [QK(10, 2)], 2: [QK(11, 2)], 4: [QK(4, 0)],
                         6: [QK(5, 0)], 8: [VG(2, 0)], 10: [VG(2, 1)],
                         12: [VG(2, 2)], 14: [VG(2, 3)]},
                # quad 2 sweep: remaining prefetch, then transposes + proj
                (2, 0): {0: [QK(10, 3)], 1: [QK(11, 3)], 2: [VG(2, 4)],
                         3: [VG(2, 5)], 4: [VG(2, 6)], 5: [VG(2, 7)],
                         6: [VG(2, 8)], 7: [VG(2, 9)], 8: [VG(2, 10)],
                         9: [VG(2, 11)], 10: [VG(2, 12)], 11: [VG(2, 13)],
                         12: [VG(2, 14), QK(4, 1)], 13: [VG(2, 15), QK(5, 1)]},
                (2, 1): {0: [QK(4, 2)], 1: [QK(5, 2)], 2: [TR(0)],
                         3: [PJ(0, 0)], 4: [PJ(0, 1)], 5: [TR(1)],
                         6: [PJ(1, 0)], 7: [PJ(1, 1)], 8: [TR(2)],
                         9: [PJ(2, 0)], 10: [PJ(2, 1)], 11: [TR(3)],
                         12: [PJ(3, 0)], 13: [PJ(3, 1)]},
                (2, 2): {0: [QK(4, 3)], 1: [QK(5, 3)], 2: [TR(4)],
                         3: [PJ(4, 0)], 4: [PJ(4, 1)], 5: [TR(5)],
                         6: [PJ(5, 0)], 7: [PJ(5, 1)], 8: [TR(6)],
                         9: [PJ(6, 0)], 10: [PJ(6, 1)], 11: [TR(7)],
                         12: [PJ(7, 0)], 13: [PJ(7, 1)]},
                (2, 3): {0: [TR(8)], 1: [PJ(8, 0)], 2: [PJ(8, 1)],
                         3: [TR(9)], 4: [PJ(9, 0)], 5: [PJ(9, 1)],
                         6: [TR(10)], 7: [PJ(10, 0)], 8: [PJ(10, 1)],
                         9: [TR(11)], 10: [PJ(11, 0)], 11: [PJ(11, 1)]},
            }

            # ---- attention sweep machinery -----------------------------
            acc_tiles: dict = {}

            def slot_ap(gen, slot):
                a, b_, c_ = acc_tiles[gen]
                if slot < 7:
                    return a, slot
                if slot < 14:
                    return b_, slot - 7
                return c_, slot - 14

            at_hist: dict = {}

            def emit_scores_exp(q: int, c: int, m: int):
                qsl = slice(c * QW, (c + 1) * QW)
                msl = slice(m * P, (m + 1) * P)
                ats = []
                for pp in range(2):
                    pair = 2 * q + pp
                    sc = psum_sc.tile(
                        [P, 1024], F32, tag="sc", name=f"sc{q}_{c}_{m}_{pp}"
                    )
                    # scoresT: keys msl on partitions, queries qsl on free dim;
                    # head 2p on PE rows 0-63, head 2p+1 on rows 64-127
                    nc.tensor.matmul(
                        sc[:, 0:QW],
                        lhsT=qkT[0:64, 6 + pair, msl],
                        rhs=qkT[0:64, pair, qsl],
                        start=True,
                        stop=True,
                    )
                    nc.tensor.matmul(
                        sc[:, QW:2 * QW],
                        lhsT=qkT[64:128, 6 + pair, msl],
                        rhs=qkT[64:128, pair, qsl],
                        start=True,
                        stop=True,
                    )
                    at = at_pool.tile(
                        [P, 1024], BF16, tag="at", name=f"at{q}_{c}_{m}_{pp}"
                    )
                    nc.scalar.activation(at[:], sc[:], AF.Exp, scale=SCALE)
                    ats.append(at)
                at_hist[(q, c, m)] = ats

            def emit_attnv(q: int, c: int, m: int):
                gen = (q, c)
                if m == 0:
                    acc_tiles[gen] = (
                        psum_acc.tile([P, 7, HD + 1], F32, tag="acca",
                                      name=f"acca{q}_{c}"),
                        psum_acc.tile([P, 7, HD + 1], F32, tag="accb",
                                      name=f"accb{q}_{c}"),
                        psum_acc.tile([P, 2, HD + 1], F32, tag="accc",
                                      name=f"accc{q}_{c}"),
                    )
                ats = at_hist.pop((q, c, m))
                for pp in range(2):
                    for hh in range(2):
                        for s in range(QS):
                            slot = pp * 8 + hh * 4 + s
                            tile_, idx = slot_ap(gen, slot)
                            # flipped attnV: out[q, hd+1] = at^T @ [v | 1].
                            # start=True clears has_written for the WHOLE
                            # bank, so only the first slot per bank (0/7/14)
                            # may set it; later slots' m=0 writes land on
                            # bank-cleared bytes and overwrite implicitly.
                            nc.tensor.matmul(
                                tile_[:, idx, :],
                                lhsT=ats[pp][:, hh * QW + s * P:
                                             hh * QW + (s + 1) * P],
                                rhs=v4[:, m, 4 * q + 2 * pp + hh, :],
                                start=(m == 0 and slot in (0, 7, 14)),
                                stop=(m == NT - 1),
                                skip_group_check=(slot not in (0, 7, 14)),
                            )

            def emit_normalize(q: int, c: int, per_s_hook=None):
                gen = (q, c)
                a, b_, c_ = acc_tiles[gen]
                # reciprocal of the 16 denominators (column 64 of each slot);
                # HW TensorScalarPtr has no divide ALU op
                rc = rcp_pool.tile([P, 16], F32, tag="rcp", name=f"rc{q}_{c}")
                nc.vector.reciprocal(rc[:, 0:7], a[:, :, HD])
                nc.vector.reciprocal(rc[:, 7:14], b_[:, :, HD])
                nc.vector.reciprocal(rc[:, 14:16], c_[:, :, HD])
                # s-major so each query subtile's four heads finish together
                # (lets the tail start its transposes per-subtile)
                for s in range(QS):
                    for pp in range(2):
                        for hh in range(2):
                            slot = pp * 8 + hh * 4 + s
                            head = 4 * q + 2 * pp + hh
                            nt_ = c * QS + s
                            tile_, idx = slot_ap(gen, slot)
                            nc.vector.tensor_scalar(
                                out=out_sb[:, nt_, head * HD:(head + 1) * HD],
                                in0=tile_[:, idx, 0:HD],
                                scalar1=rc[:, slot:slot + 1],
                                scalar2=None,
                                op0=ALU.mult,
                            )
                    if per_s_hook is not None:
                        per_s_hook(s)
                del acc_tiles[gen]

            # ---- main loop: software-pipelined quad-major sweeps -------
            # upfront qkT groups on the scores psum ring (attention idle).
            # ct-major across all four groups so each arriving w band
            # unblocks four matmuls instead of head-blocking the PE FIFO;
            # two groups pack per sc tile in separate banks.
            up1 = psum_sc.tile([P, 1024], F32, tag="sc", name="up1")
            up2 = psum_sc.tile([P, 1024], F32, tag="sc", name="up2")
            for ct in range(CT):
                for ps, jt in ((up1[:, 0:QW], 6), (up1[:, QW:2 * QW], 0),
                               (up2[:, 0:QW], 7), (up2[:, QW:2 * QW], 1)):
                    nc.tensor.matmul(
                        ps,
                        lhsT=w_qkv_sb[:, ct, jt * P:(jt + 1) * P],
                        rhs=xT[:, ct, 0:QW],
                        start=(ct == 0),
                        stop=(ct == CT - 1),
                    )
            nc.vector.tensor_copy(out=qkT[:, 6, 0:QW], in_=up1[:, 0:QW])
            nc.vector.tensor_copy(out=qkT[:, 0, 0:QW], in_=up1[:, QW:2 * QW])
            nc.vector.tensor_copy(out=qkT[:, 7, 0:QW], in_=up2[:, 0:QW])
            nc.vector.tensor_copy(out=qkT[:, 1, 0:QW], in_=up2[:, QW:2 * QW])

            iters = [(q, c, m) for q in range(3) for c in range(NCH)
                     for m in range(NT)]
            emit_scores_exp(*iters[0])
            for i, (q, c, m) in enumerate(iters):
                # issue next iteration's scores first so ScalarE never waits
                if i + 1 < len(iters):
                    emit_scores_exp(*iters[i + 1])
                # spare tasks precede attnV: v-group m must be defined before
                # the attnV that consumes it (program order = PE queue order)
                for t in plan.get((q, c), {}).get(m, ()):
                    t()
                emit_attnv(q, c, m)
                if m == NT - 1:
                    if (q, c) == (2, NCH - 1):
                        # tail window: transpose each subtile the moment its
                        # last four heads normalize (sc ring is free now) and
                        # chase each transpose with the previous subtile's
                        # projections so the PE never drains
                        def tail_tr(s):
                            tp = psum_sc.tile([P, 8, P], BF16, tag="sc",
                                              name=f"tp{12 + s}")
                            nt_ = 12 + s
                            for ct in range(CT):
                                nc.tensor.transpose(
                                    tp[:, ct, :],
                                    out_sb[:, nt_, ct * P:(ct + 1) * P], ident
                                )
                            nsl = slice(nt_ * P, (nt_ + 1) * P)
                            nc.vector.tensor_copy(out=outT[:, 0:4, nsl],
                                                  in_=tp[:, 0:4, :])
                            nc.vector.tensor_copy(out=outT[:, 4:6, nsl],
                                                  in_=tp[:, 4:6, :])
                            if s >= 1:
                                emit_proj(11 + s, 0, psum_sp, "sp")
                                emit_proj(11 + s, 1, psum_sc, "sc")

                        emit_normalize(q, c, per_s_hook=tail_tr)
                    else:
                        emit_normalize(q, c)

            # ---- tail: last projections --------------------------------
            emit_proj(15, 0, psum_sp, "sp")
            emit_proj(15, 1, psum_sc, "sc")

            _DBG_TILES.update(
                qkT=qkT, v4=v4, out_sb=out_sb, outT=outT, xT=xT,
                w_qkv_sb=w_qkv_sb,
            )

    nc.compile()
    return nc


_DBG_TILES: dict = {}


_NC_CACHE: list = []


def _get_nc() -> bass.Bass:
    if not _NC_CACHE:
        _NC_CACHE.append(build_nc())
    return _NC_CACHE[0]


def run(inputs: dict, trace: bool = False):
    """Run on 8 NeuronCores.  Returns (out [B,N,C] f32, exec_time_ns|None)."""
    nc = _get_nc()
    x = np.ascontiguousarray(np.asarray(inputs["x"], dtype=np.float32))
    w_qkv = np.ascontiguousarray(np.asarray(inputs["w_qkv"], dtype=np.float32))
    w_proj = np.ascontiguousarray(np.asarray(inputs["w_proj"], dtype=np.float32))
    b_proj = np.ascontiguousarray(np.asarray(inputs["b_proj"], dtype=np.float32))
    in_maps = [
        {"x": x[i], "w_qkv": w_qkv, "w_proj": w_proj, "b_proj": b_proj}
        for i in range(B)
    ]
    try:
        res = bass_utils.run_bass_kernel_spmd(
            nc, in_maps, core_ids=list(range(B)), trace=trace
        )
    except ModuleNotFoundError:
        res = bass_utils.run_bass_kernel_spmd(
            nc, in_maps, core_ids=list(range(B)), trace=False
        )
    out = np.stack([res.results[i]["out"] for i in range(B)], axis=0)
    return out.astype(np.float32), res.exec_time_ns


def kernel(x, w_qkv, w_proj, b_proj):
    trace = os.environ.get("BASS_KERNEL_TRACE", "0") == "1"
    out, _ = run(
        {"x": x, "w_qkv": w_qkv, "w_proj": w_proj, "b_proj": b_proj}, trace=trace
    )
    return out


# revision 24
# speedup vs baseline: 1.0159x; 1.0159x over previous
"""Trainium2 Bass kernel v2 for multi-head attention (dense transformer).

Reference computation (per batch element):
    qkv = x @ w_qkv                      # [N, 3C]
    q, k, v = split heads (H=12, HD=64); q *= HD**-0.5
    out = softmax(q k^T) v               # full [N, N] scores
    out = merge_heads(out) @ w_proj + b_proj

Distribution: pure data parallel over batch — 8 elements, 8 cores.

Cost-model-driven redesign vs v1: the CoreSim cost model charges a matmul
out_free_dim x 0.42ns regardless of K/M, so the v1 denominator matmuls
(768 x FD=512) cost a full 166us and the col-packed attnV another 166us.
v2 restructures:
  * attnV is FLIPPED: lhsT = at-tile [128 keys, 128 queries] (stationary),
    rhs = v_ext [128 keys, 65] where column 64 is ones -> out [128 q, 65]
    accumulated over the 16 key tiles.  M=128 doubles the output per
    instruction (FD=65 vs 512), and the softmax denominator rides along as
    column 64 for +1 cycle.  attnV+denominators: 332us -> 83us.
  * normalization becomes a per-partition tensor_scalar DIVIDE on DVE
    (queries live on partitions), killing v1's DMA-bounce broadcast.
  * out is produced in [token, channel] layout; 96 PE transposes (~5us)
    restore the lhsT layout for the final projection.
  * sweep order is quad-major (3 sweeps of 4 heads x full N), so the
    qkT/v prefetch for quad g+1 hides inside quad g's ACT-bound sweep and
    the proj/transpose work hides inside quad 2's sweep.
ScalarE (384 x FD=1024 exp, ~399us busy) is the design floor; everything
else is scheduled to keep it saturated.

PSUM budget (8 banks): scores 2x[128,1024]f32 (4) + attnV accum 7/7/2
slots of [128,65]f32 (3) + one shared spare bank for qk/v/proj/transpose
groups (1).
"""

import os

import numpy as np

import concourse.bass as bass
import concourse.mybir as mybir
from concourse import bacc, bass_utils
from concourse.masks import make_identity
from concourse.tile import TileContext

F32 = mybir.dt.float32
BF16 = mybir.dt.bfloat16
AF = mybir.ActivationFunctionType
ALU = mybir.AluOpType

B, N, C = 8, 2048, 768
H, HD = 12, 64
SCALE = HD ** -0.5  # folded into the exp activation
P = 128
NT = N // P          # 16 key tiles
CT = C // P          # 6 feature tiles
NCH = 4              # query chunks per sweep
QW = N // NCH        # 512
QS = QW // P         # 4 query subtiles per chunk


def build_nc() -> bass.Bass:
    nc = bacc.Bacc(None)
    x = nc.declare_dram_parameter("x", [N, C], F32, isOutput=False)
    w_qkv = nc.declare_dram_parameter("w_qkv", [C, 3 * C], F32, isOutput=False)
    w_proj = nc.declare_dram_parameter("w_proj", [C, C], F32, isOutput=False)
    b_proj = nc.declare_dram_parameter("b_proj", [C], F32, isOutput=False)
    out = nc.declare_dram_parameter("out", [N, C], F32, isOutput=True)

    with TileContext(nc) as tc:
        with (
            tc.tile_pool(name="const", bufs=1) as cpool,
            tc.tile_pool(name="dram", bufs=1, space="DRAM") as dpool,
            tc.tile_pool(name="at", bufs=10) as at_pool,
            tc.tile_pool(name="rcp", bufs=2) as rcp_pool,
            tc.tile_pool(name="fin", bufs=2) as fin_pool,
            tc.tile_pool(name="psc", bufs=2, space="PSUM") as psum_sc,
            tc.tile_pool(name="pacc", bufs=1, space="PSUM") as psum_acc,
            tc.tile_pool(name="psp", bufs=1, space="PSUM") as psum_sp,
        ):
            # ---- persistent SBUF tensors -------------------------------
            w_qkv_sb = cpool.tile([P, CT, 3 * C], BF16, tag="wqkv")
            wproj_sb = cpool.tile([P, CT, C], BF16, tag="wproj")
            b_bc = cpool.tile([P, C], F32, tag="bias")
            xT = cpool.tile([P, CT, N], BF16, tag="xT")
            qkT = cpool.tile([P, 12, N], BF16, tag="qkT")  # q pairs 0-5, k 6-11
            v4 = cpool.tile([P, NT, H, HD + 1], BF16, tag="v4")  # col 64 = ones
            out_sb = cpool.tile([P, NT, C], BF16, tag="outsb")  # [token, chan]
            outT = cpool.tile([P, CT, N], BF16, tag="outT")
            ident = cpool.tile([P, P], BF16, tag="ident")
            warm = cpool.tile([P, 1], F32, tag="warm")

            # ---- phase 0: act-table preload, input DMAs ----------------
            nc.vector.memset(warm[:], 0.0)
            nc.scalar.activation(warm[:], warm[:], AF.Exp)  # preload exp table
            make_identity(nc, ident)
            nc.vector.memset(v4[:, :, :, HD], 1.0)  # denominator ones columns

            # DMA transfers serialize on a single global device in the cost
            # model, so the startup chain is ordered by need: x casts, the
            # first quarter of the xT transposes (token chunk 0), the w_qkv
            # bands (gate the upfront qkT groups), then the remaining
            # transpose pieces ahead of their kT/qT deadlines.
            x_bf = dpool.tile([N, C], BF16)
            wq_r = w_qkv.rearrange("(o p) j -> p o j", p=P)
            for ct in range(CT):
                csl = slice(ct * P, (ct + 1) * P)
                nc.gpsimd.dma_start(out=x_bf[:, csl], in_=x[:, csl])
            # the four upfront-needed qk column blocks first (small DMAs),
            # then the rest in three block loads off the critical path
            for jt in (6, 0, 7, 1):
                nc.gpsimd.dma_start(
                    out=w_qkv_sb[:, :, jt * P:(jt + 1) * P],
                    in_=wq_r[:, :, jt * P:(jt + 1) * P],
                )
            nc.gpsimd.dma_start(
                out=w_qkv_sb[:, :, 12 * P:18 * P], in_=wq_r[:, :, 12 * P:18 * P]
            )
            nc.gpsimd.dma_start(
                out=w_qkv_sb[:, :, 2 * P:6 * P], in_=wq_r[:, :, 2 * P:6 * P]
            )
            nc.gpsimd.dma_start(
                out=w_qkv_sb[:, :, 8 * P:12 * P], in_=wq_r[:, :, 8 * P:12 * P]
            )
            for ct in range(CT):
                csl = slice(ct * P, (ct + 1) * P)
                nc.sync.dma_start_transpose(xT[:, ct, :], x_bf[:, csl])
            nc.sync.dma_start(out=b_bc[:], in_=b_proj[None, :].to_broadcast((P, C)))
            nc.gpsimd.dma_start(
                out=wproj_sb[:], in_=w_proj.rearrange("(o p) j -> p o j", p=P)
            )

            # ---- emit helpers ------------------------------------------
            def emit_qk_group(jt: int, c4: int, pool, tag: str):
                """qkT[:, jt, c4*512:...] = w_qk(jt)^T @ x^T chunk."""
                ps = pool.tile([P, QW], F32, tag=tag, name=f"qk{jt}_{c4}")
                for ct in range(CT):
                    nc.tensor.matmul(
                        ps[:, 0:QW],
                        lhsT=w_qkv_sb[:, ct, jt * P:(jt + 1) * P],
                        rhs=xT[:, ct, c4 * QW:(c4 + 1) * QW],
                        start=(ct == 0),
                        stop=(ct == CT - 1),
                    )
                nc.vector.tensor_copy(
                    out=qkT[:, jt, c4 * QW:(c4 + 1) * QW], in_=ps[:, 0:QW]
                )

            def emit_v_group(m: int, g: int):
                """v for key tile m, heads 4g..4g+3, natural [key, hd] layout."""
                ps = psum_sp.tile([P, QW], F32, tag="sp", name=f"v{g}_{m}")
                for ct in range(CT):
                    nc.tensor.matmul(
                        ps[:, 0:256],
                        lhsT=xT[:, ct, m * P:(m + 1) * P],
                        rhs=w_qkv_sb[:, ct, 2 * C + g * 256: 2 * C + (g + 1) * 256],
                        start=(ct == 0),
                        stop=(ct == CT - 1),
                    )
                nc.vector.tensor_copy(
                    out=v4[:, m, 4 * g:4 * (g + 1), 0:HD], in_=ps[:, 0:256]
                )

            def emit_transpose(nt_: int):
                """outT[:, :, nt] = out_sb[nt]^T via 6 PE-mode transposes."""
                tp = psum_sp.tile([P, 8, P], BF16, tag="sp", name=f"tp{nt_}")
                for ct in range(CT):
                    nc.tensor.transpose(
                        tp[:, ct, :], out_sb[:, nt_, ct * P:(ct + 1) * P], ident
                    )
                nsl = slice(nt_ * P, (nt_ + 1) * P)
                nc.vector.tensor_copy(out=outT[:, 0:4, nsl], in_=tp[:, 0:4, :])
                nc.vector.tensor_copy(out=outT[:, 4:6, nsl], in_=tp[:, 4:6, :])

            def emit_proj(nt_: int, slot_: int, pool, tag: str):
                """final[nt tile, eo:eo+ew] = outT^T @ w_proj + b."""
                eo, ew = ((0, QW), (QW, 256))[slot_]
                ps = pool.tile([P, QW], F32, tag=tag, name=f"pj{nt_}_{slot_}")
                for ct in range(CT):
                    nc.tensor.matmul(
                        ps[:, 0:ew],
                        lhsT=outT[:, ct, nt_ * P:(nt_ + 1) * P],
                        rhs=wproj_sb[:, ct, eo:eo + ew],
                        start=(ct == 0),
                        stop=(ct == CT - 1),
                    )
                fs = fin_pool.tile([P, QW], F32, tag="fin")
                nc.vector.tensor_tensor(
                    fs[:, 0:ew], ps[:, 0:ew], b_bc[:, eo:eo + ew], ALU.add
                )
                nc.sync.dma_start(
                    out=out[nt_ * P:(nt_ + 1) * P, eo:eo + ew], in_=fs[:, 0:ew]
                )

            # ---- spare-bank task plan per (quad, chunk) window ---------
            def QK(jt, c4):
                return lambda: emit_qk_group(jt, c4, psum_sp, "sp")

            def VG(g, m):
                return lambda: emit_v_group(m, g)

            def TR(nt_):
                return lambda: emit_transpose(nt_)

            def PJ(nt_, s_):
                return lambda: emit_proj(nt_, s_, psum_sp, "sp")

            # plan[(q, c)][m] = spare tasks emitted at iteration (q, c, m),
            # placed BEFORE that iteration's attnV and AFTER its lookahead
            # scores.  Deadlines: VG(g, m') by iteration m' of g's first
            # chunk; kT QK(jt, c4) one iteration before scores m=4*c4; next
            # chunk's qT one iteration before the window ends.
            plan = {
                # quad 0 sweep: own kT/qT tails + v heads 0-3; prefetch quad 1
                (0, 0): {0: [VG(0, 0)], 1: [VG(0, 1), QK(6, 1), QK(7, 1)],
                         2: [VG(0, 2)], 3: [VG(0, 3)],
                         4: [VG(0, 4), QK(6, 2)], 5: [VG(0, 5), QK(7, 2)],
                         6: [VG(0, 6)], 7: [VG(0, 7), QK(6, 3)],
                         8: [VG(0, 8)], 9: [VG(0, 9), QK(7, 3)],
                         10: [VG(0, 10)], 11: [VG(0, 11), QK(0, 1)],
                         12: [VG(0, 12)], 13: [VG(0, 13), QK(1, 1)],
                         14: [VG(0, 14)], 15: [VG(0, 15)]},
                (0, 1): {0: [QK(0, 2)], 2: [QK(1, 2)], 4: [QK(8, 0)],
                         6: [QK(9, 0)]},
                (0, 2): {0: [QK(0, 3)], 2: [QK(1, 3)], 4: [QK(8, 1)],
                         6: [QK(9, 1)]},
                (0, 3): {0: [QK(8, 2)], 2: [QK(9, 2)], 4: [QK(2, 0)],
                         6: [QK(3, 0)], 8: [VG(1, 0)], 10: [VG(1, 1)],
                         12: [VG(1, 2)], 14: [VG(1, 3)]},
                # quad 1 sweep
                (1, 0): {0: [QK(8, 3)], 1: [QK(9, 3)], 2: [VG(1, 4)],
                         3: [VG(1, 5)], 4: [VG(1, 6)], 5: [VG(1, 7)],
                         6: [VG(1, 8)], 7: [VG(1, 9)], 8: [VG(1, 10)],
                         9: [VG(1, 11)], 10: [VG(1, 12)], 11: [VG(1, 13)],
                         12: [VG(1, 14), QK(2, 1)], 13: [VG(1, 15), QK(3, 1)]},
                (1, 1): {0: [QK(2, 2)], 2: [QK(3, 2)], 4: [QK(10, 0)],
                         6: [QK(11, 0)]},
                (1, 2): {0: [QK(2, 3)], 2: [QK(3, 3)], 4: [QK(10, 1)],
                         6: [QK(11, 1)]},
                (1, 3): {0: [QK(10, 2)], 2: [QK(11, 2)], 4: [QK(4, 0)],
                         6: [QK(5, 0)], 8: [VG(2, 0)], 10: [VG(2, 1)],
                         12: [VG(2, 2)], 14: [VG(2, 3)]},
                # quad 2 sweep: remaining prefetch, then transposes + proj
                (2, 0): {0: [QK(10, 3)], 1: [QK(11, 3)], 2: [VG(2, 4)],
                         3: [VG(2, 5)], 4: [VG(2, 6)], 5: [VG(2, 7)],
                         6: [VG(2, 8)], 7: [VG(2, 9)], 8: [VG(2, 10)],
                         9: [VG(2, 11)], 10: [VG(2, 12)], 11: [VG(2, 13)],
                         12: [VG(2, 14), QK(4, 1)], 13: [VG(2, 15), QK(5, 1)]},
                (2, 1): {0: [QK(4, 2)], 1: [QK(5, 2)], 2: [TR(0)],
                         3: [PJ(0, 0)], 4: [PJ(0, 1)], 5: [TR(1)],
                         6: [PJ(1, 0)], 7: [PJ(1, 1)], 8: [TR(2)],
                         9: [PJ(2, 0)], 10: [PJ(2, 1)], 11: [TR(3)],
                         12: [PJ(3, 0)], 13: [PJ(3, 1)]},
                (2, 2): {0: [QK(4, 3)], 1: [QK(5, 3)], 2: [TR(4)],
                         3: [PJ(4, 0)], 4: [PJ(4, 1)], 5: [TR(5)],
                         6: [PJ(5, 0)], 7: [PJ(5, 1)], 8: [TR(6)],
                         9: [PJ(6, 0)], 10: [PJ(6, 1)], 11: [TR(7)],
                         12: [PJ(7, 0)], 13: [PJ(7, 1)]},
                (2, 3): {0: [TR(8)], 1: [PJ(8, 0)], 2: [PJ(8, 1)],
                         3: [TR(9)], 4: [PJ(9, 0)], 5: [PJ(9, 1)],
                         6: [TR(10)], 7: [PJ(10, 0)], 8: [PJ(10, 1)],
                         9: [TR(11)], 10: [PJ(11, 0)], 11: [PJ(11, 1)]},
            }

            # ---- attention sweep machinery -----------------------------
            acc_tiles: dict = {}

            def slot_ap(gen, slot):
                a, b_, c_ = acc_tiles[gen]
                if slot < 7:
                    return a, slot
                if slot < 14:
                    return b_, slot - 7
                return c_, slot - 14

            at_hist: dict = {}

            def emit_scores_exp(q: int, c: int, m: int):
                qsl = slice(c * QW, (c + 1) * QW)
                msl = slice(m * P, (m + 1) * P)
                ats = []
                for pp in range(2):
                    pair = 2 * q + pp
                    sc = psum_sc.tile(
                        [P, 1024], F32, tag="sc", name=f"sc{q}_{c}_{m}_{pp}"
                    )
                    # scoresT: keys msl on partitions, queries qsl on free dim;
                    # head 2p on PE rows 0-63, head 2p+1 on rows 64-127
                    nc.tensor.matmul(
                        sc[:, 0:QW],
                        lhsT=qkT[0:64, 6 + pair, msl],
                        rhs=qkT[0:64, pair, qsl],
                        start=True,
                        stop=True,
                    )
                    nc.tensor.matmul(
                        sc[:, QW:2 * QW],
                        lhsT=qkT[64:128, 6 + pair, msl],
                        rhs=qkT[64:128, pair, qsl],
                        start=True,
                        stop=True,
                    )
                    at = at_pool.tile(
                        [P, 1024], BF16, tag="at", name=f"at{q}_{c}_{m}_{pp}"
                    )
                    nc.scalar.activation(at[:], sc[:], AF.Exp, scale=SCALE)
                    ats.append(at)
                at_hist[(q, c, m)] = ats

            def emit_attnv(q: int, c: int, m: int):
                gen = (q, c)
                if m == 0:
                    acc_tiles[gen] = (
                        psum_acc.tile([P, 7, HD + 1], F32, tag="acca",
                                      name=f"acca{q}_{c}"),
                        psum_acc.tile([P, 7, HD + 1], F32, tag="accb",
                                      name=f"accb{q}_{c}"),
                        psum_acc.tile([P, 2, HD + 1], F32, tag="accc",
                                      name=f"accc{q}_{c}"),
                    )
                ats = at_hist.pop((q, c, m))
                for pp in range(2):
                    for hh in range(2):
                        for s in range(QS):
                            slot = pp * 8 + hh * 4 + s
                            tile_, idx = slot_ap(gen, slot)
                            # flipped attnV: out[q, hd+1] = at^T @ [v | 1].
                            # start=True clears has_written for the WHOLE
                            # bank, so only the first slot per bank (0/7/14)
                            # may set it; later slots' m=0 writes land on
                            # bank-cleared bytes and overwrite implicitly.
                            nc.tensor.matmul(
                                tile_[:, idx, :],
                                lhsT=ats[pp][:, hh * QW + s * P:
                                             hh * QW + (s + 1) * P],
                                rhs=v4[:, m, 4 * q + 2 * pp + hh, :],
                                start=(m == 0 and slot in (0, 7, 14)),
                                stop=(m == NT - 1),
                                skip_group_check=(slot not in (0, 7, 14)),
                            )

            def emit_normalize(q: int, c: int, per_s_hook=None):
                gen = (q, c)
                a, b_, c_ = acc_tiles[gen]
                # reciprocal of the 16 denominators (column 64 of each slot);
                # HW TensorScalarPtr has no divide ALU op
                rc = rcp_pool.tile([P, 16], F32, tag="rcp", name=f"rc{q}_{c}")
                nc.vector.reciprocal(rc[:, 0:7], a[:, :, HD])
                nc.vector.reciprocal(rc[:, 7:14], b_[:, :, HD])
                nc.vector.reciprocal(rc[:, 14:16], c_[:, :, HD])
                # s-major so each query subtile's four heads finish together
                # (lets the tail start its transposes per-subtile)
                for s in range(QS):
                    for pp in range(2):
                        for hh in range(2):
                            slot = pp * 8 + hh * 4 + s
                            head = 4 * q + 2 * pp + hh
                            nt_ = c * QS + s
                            tile_, idx = slot_ap(gen, slot)
                            nc.vector.tensor_scalar(
                                out=out_sb[:, nt_, head * HD:(head + 1) * HD],
                                in0=tile_[:, idx, 0:HD],
                                scalar1=rc[:, slot:slot + 1],
                                scalar2=None,
                                op0=ALU.mult,
                            )
                    if per_s_hook is not None:
                        per_s_hook(s)
                del acc_tiles[gen]

            # ---- main loop: software-pipelined quad-major sweeps -------
            # upfront qkT groups on the scores psum ring (attention idle).
            # ct-major across all four groups so each arriving w band
            # unblocks four matmuls instead of head-blocking the PE FIFO;
            # two groups pack per sc tile in separate banks.
            up1 = psum_sc.tile([P, 1024], F32, tag="sc", name="up1")
            up2 = psum_sc.tile([P, 1024], F32, tag="sc", name="up2")
            for ct in range(CT):
                for ps, jt in ((up1[:, 0:QW], 6), (up1[:, QW:2 * QW], 0),
                               (up2[:, 0:QW], 7), (up2[:, QW:2 * QW], 1)):
                    nc.tensor.matmul(
                        ps,
                        lhsT=w_qkv_sb[:, ct, jt * P:(jt + 1) * P],
                        rhs=xT[:, ct, 0:QW],
                        start=(ct == 0),
                        stop=(ct == CT - 1),
                    )
            nc.vector.tensor_copy(out=qkT[:, 6, 0:QW], in_=up1[:, 0:QW])
            nc.vector.tensor_copy(out=qkT[:, 0, 0:QW], in_=up1[:, QW:2 * QW])
            nc.vector.tensor_copy(out=qkT[:, 7, 0:QW], in_=up2[:, 0:QW])
            nc.vector.tensor_copy(out=qkT[:, 1, 0:QW], in_=up2[:, QW:2 * QW])

            iters = [(q, c, m) for q in range(3) for c in range(NCH)
                     for m in range(NT)]
            emit_scores_exp(*iters[0])
            for i, (q, c, m) in enumerate(iters):
                # issue next iteration's scores first so ScalarE never waits
                if i + 1 < len(iters):
                    emit_scores_exp(*iters[i + 1])
                # spare tasks precede attnV: v-group m must be defined before
                # the attnV that consumes it (program order = PE queue order)
                for t in plan.get((q, c), {}).get(m, ()):
                    t()
                emit_attnv(q, c, m)
                if m == NT - 1:
                    if (q, c) == (2, NCH - 1):
                        # tail window: transpose each subtile the moment its
                        # last four heads normalize (sc ring is free now) and
                        # chase each transpose with the previous subtile's
                        # projections so the PE never drains
                        def tail_tr(s):
                            tp = psum_sc.tile([P, 8, P], BF16, tag="sc",
                                              name=f"tp{12 + s}")
                            nt_ = 12 + s
                            for ct in range(CT):
                                nc.tensor.transpose(
                                    tp[:, ct, :],
                                    out_sb[:, nt_, ct * P:(ct + 1) * P], ident
                                )
                            nsl = slice(nt_ * P, (nt_ + 1) * P)
                            nc.vector.tensor_copy(out=outT[:, 0:4, nsl],
                                                  in_=tp[:, 0:4, :])
                            nc.vector.tensor_copy(out=outT[:, 4:6, nsl],
                                                  in_=tp[:, 4:6, :])
                            if s >= 1:
                                emit_proj(11 + s, 0, psum_sp, "sp")
                                emit_proj(11 + s, 1, psum_sc, "sc")

                        emit_normalize(q, c, per_s_hook=tail_tr)
                    else:
                        emit_normalize(q, c)

            # ---- tail: last projections --------------------------------
            emit_proj(15, 0, psum_sp, "sp")
            emit_proj(15, 1, psum_sc, "sc")

            _DBG_TILES.update(
                qkT=qkT, v4=v4, out_sb=out_sb, outT=outT, xT=xT,
                w_qkv_sb=w_qkv_sb,
            )

    nc.compile()
    return nc


_DBG_TILES: dict = {}


_NC_CACHE: list = []


def _get_nc() -> bass.Bass:
    if not _NC_CACHE:
        _NC_CACHE.append(build_nc())
    return _NC_CACHE[0]


def run(inputs: dict, trace: bool = False):
    """Run on 8 NeuronCores.  Returns (out [B,N,C] f32, exec_time_ns|None)."""
    nc = _get_nc()
    x = np.ascontiguousarray(np.asarray(inputs["x"], dtype=np.float32))
    w_qkv = np.ascontiguousarray(np.asarray(inputs["w_qkv"], dtype=np.float32))
    w_proj = np.ascontiguousarray(np.asarray(inputs["w_proj"], dtype=np.float32))
    b_proj = np.ascontiguousarray(np.asarray(inputs["b_proj"], dtype=np.float32))
    in_maps = [
        {"x": x[i], "w_qkv": w_qkv, "w_proj": w_proj, "b_proj": b_proj}
        for i in range(B)
    ]
    try:
        res = bass_utils.run_bass_kernel_spmd(
            nc, in_maps, core_ids=list(range(B)), trace=trace
        )
    except ModuleNotFoundError:
        res = bass_utils.run_bass_kernel_spmd(
            nc, in_maps, core_ids=list(range(B)), trace=False
        )
    out = np.stack([res.results[i]["out"] for i in range(B)], axis=0)
    return out.astype(np.float32), res.exec_time_ns


def kernel(x, w_qkv, w_proj, b_proj):
    trace = os.environ.get("BASS_KERNEL_TRACE", "0") == "1"
    out, _ = run(
        {"x": x, "w_qkv": w_qkv, "w_proj": w_proj, "b_proj": b_proj}, trace=trace
    )
    return out
